# revision 13
# baseline (speedup 1.0000x reference)
"""CrossHazardInteractionLayer TRN2 kernel (v5).

Data-parallel over batch B=8 -> 8 NeuronCores (one batch element each).
Host prep: fold the |M|>thr gate into W2 (pre-scaled), cast W1/W2/x to
bf16, transpose x to feature-major (d on partitions) once.  Device:
  stage 1 per source s: hT[(t,k), n] = gelu(x[s]^T-tiles @ W1[s,:]) for
    all active targets, packed 2 targets per 128-col chunk; exact-erf
    GELU fused into the PSUM->SBUF copy on the scalar engine.
  stage 2 per target t: addT[d, n] = sum over (s,k) j-tiles of
    (gate*W2)^T-stationary @ hT-moving, written bf16 d-major.  The
    x residual is added on the HOST in assemble() (it is pure O(N)
    data movement; keeping it off-device halves the DVE drain load and
    lets the PSUM->SBUF cast split between scalar and vector engines).

Trace-driven scheduling rules (vs the 251us v2 baseline):
  - Each engine owns ~4 DMA-completion semaphore slots; a 5th doorbell
    BLOCKS the engine until an earlier DMA completes.  The scalar
    engine runs the latency-critical GELUs, so it rings only a 3-door
    startup prefix (b1 + 2 w2 tiles); everything else rides sync/gpsimd
    (whole-tile transfers, 0.6-0.8MB each for DMA efficiency).
  - w2 is prefetched during pass-0 stage 1 (baseline parked it behind
    x tiles on gpsimd -> 8us of PE stalls + a HAM re-throttle).
  - x for pass p+1 is prefetched at the TOP of pass p.
  - stage-2 partial k-tiles (targets with odd #sources) are padded to
    full 128 rows (w2 rows zero-padded host-side, h tail rows memset)
    so their LDWEIGHTS pipeline through the background weight buffer.
  - stage-2 targets run in h-availability order; PSUM->SBUF copies
    alternate scalar/vector; stores are one fat DMA per target except
    the very last target, which stores per-o-pair to shrink the tail.
"""

import numpy as np
import ml_dtypes

import concourse.bass as bass
import concourse.mybir as mybir
import concourse.tile as tile
from concourse import bacc

H = 7
B = 8
S = 2048
D = 768
K = 64
P = 128
PASS = 512          # seq cols per pass
NPASS = S // PASS
DT = D // P         # d-tiles (6)
THR = 0.05

F32 = mybir.dt.float32
BF16 = mybir.dt.bfloat16
GELU = mybir.ActivationFunctionType.Gelu
COPY = mybir.ActivationFunctionType.Copy

_CACHE: dict = {}


def _build(has_b2=False, act_t=None, act_s=None):
    """act_t[s] = tuple of active targets for source s (packing order);
    act_s[t] = tuple of active sources for target t (packing order)."""
    import math
    if act_t is None:
        act_t = tuple(tuple(t for t in range(H) if t != s) for s in range(H))
    if act_s is None:
        act_s = tuple(tuple(s for s in range(H) if s != t) for t in range(H))
    s2tiles = [math.ceil(len(act_s[t]) * K / P) for t in range(H)]
    # targets whose last k-tile is only half-filled (padded to 128 rows)
    s2pad = [t for t in range(H) if len(act_s[t]) and (len(act_s[t]) * K) % P]

    # odd-tail pairing for stage 1
    fullc = [len(act_t[s]) * K // P for s in range(H)]
    odd = [s for s in range(H) if (len(act_t[s]) * K) % P]
    pair_after = {}
    for i in range(0, len(odd) - 1, 2):
        pair_after[odd[i + 1]] = (odd[i], odd[i + 1])
    leftover = odd[-1] if len(odd) % 2 else None

    # stage-1 chunk emission index of each (s,t) GELU -> stage-2 target order
    emit_idx = {}
    ctr = 0
    for s in range(H):
        for mc in range(fullc[s]):
            emit_idx[(s, act_t[s][2 * mc])] = ctr
            emit_idx[(s, act_t[s][2 * mc + 1])] = ctr
            ctr += 1
        if s == leftover:
            emit_idx[(s, act_t[s][2 * fullc[s]])] = ctr
            ctr += 1
        if s in pair_after:
            sA, sB = pair_after[s]
            emit_idx[(sA, act_t[sA][2 * fullc[sA]])] = ctr
            emit_idx[(sB, act_t[sB][2 * fullc[sB]])] = ctr
            ctr += 1
    t_last = {t: max([emit_idx[(s, t)] for s in act_s[t]] or [-1])
              for t in range(H)}
    t_order = sorted(range(H), key=lambda t: (t_last[t], t))

    nc = bacc.Bacc("TRN2", target_bir_lowering=False, debug=False)
    xtd = nc.declare_dram_parameter("xtd", [H, NPASS, P, DT, PASS], BF16, isOutput=False)
    w1h = nc.declare_dram_parameter("w1h", [H, P, DT, 3 * P], BF16, isOutput=False)
    w2h = nc.declare_dram_parameter("w2h", [H, P, 3, D], BF16, isOutput=False)
    b1h = nc.declare_dram_parameter("b1h", [P, 4, H], F32, isOutput=False)
    b2h = nc.declare_dram_parameter("b2h", [P, DT, H], F32, isOutput=False)
    outd = nc.declare_dram_parameter("outd", [H, NPASS, P, DT, PASS], BF16, isOutput=True)

    with tile.TileContext(nc) as tc:
        with tc.tile_pool(name="static", bufs=1) as st, \
             tc.tile_pool(name="xt", bufs=2) as xtp, \
             tc.tile_pool(name="ht", bufs=2) as htp, \
             tc.tile_pool(name="osb", bufs=2) as osp, \
             tc.tile_pool(name="s1_ps", bufs=4, space="PSUM") as s1p, \
             tc.tile_pool(name="s2_ps", bufs=2, space="PSUM") as s2p:

            b1sb = st.tile([P, 4, H], F32, tag="b1sb")
            nc.scalar.dma_start(b1sb[:], b1h[:])
            w1sb = [None] * H
            w2sb = [None] * H
            b2sb = None

            xts = [[None] * H for _ in range(2)]

            def load_x(p):
                dst = xts[p % 2]
                for s in range(H):
                    xt = xtp.tile([P, DT, PASS], BF16, tag=f"xt{s}")
                    eng = [nc.sync, nc.gpsimd][(s + p) % 2]
                    eng.dma_start(xt[:], xtd[s, p])
                    dst[s] = xt

            # ---- passes over the sequence ----
            for p in range(NPASS):
                if p == 0:
                    # startup: whole-tile w1/x alternating sync/gpsimd in
                    # consumption order; first 2 w2 tiles on scalar's free
                    # startup slots, the rest behind the pass-0 tiles
                    for s in range(H):
                        w = st.tile([P, DT, 3 * P], BF16, tag=f"w1_{s}",
                                    name=f"w1_{s}")
                        [nc.sync, nc.gpsimd][s % 2].dma_start(w[:], w1h[s])
                        w1sb[s] = w
                        xt = xtp.tile([P, DT, PASS], BF16, tag=f"xt{s}")
                        [nc.gpsimd, nc.sync][s % 2].dma_start(xt[:], xtd[s, 0])
                        xts[0][s] = xt
                    if has_b2:
                        b2sb = st.tile([P, DT, H], F32, tag="b2sb")
                        nc.gpsimd.dma_start(b2sb[:], b2h[:])
                    nw2 = 0
                    for t in t_order:
                        if not act_s[t]:
                            continue
                        w = st.tile([P, 3, D], BF16, tag=f"w2_{t}", name=f"w2_{t}")
                        eng = nc.scalar if nw2 < 2 else \
                            [nc.sync, nc.gpsimd][nw2 % 2]
                        eng.dma_start(w[:], w2h[t])
                        w2sb[t] = w
                        nw2 += 1
                if p + 1 < NPASS:
                    load_x(p + 1)
                cur = xts[p % 2]

                hts = [htp.tile([P, 3, PASS], BF16, tag=f"ht{t}", name=f"ht{t}")
                       for t in range(H)]
                # zero the never-written tail rows of half-filled k-tiles so
                # the padded full-row stage-2 matmuls multiply 0-weights by
                # finite values
                for t in s2pad:
                    r0 = (len(act_s[t]) * K) % P
                    nc.vector.memset(hts[t][r0:, s2tiles[t] - 1, :], 0.0)

                # stage 1
                def s1_act(ss, half, ps1, bias_slot):
                    t = act_t[ss][2 * fullc[ss]] if bias_slot >= fullc[ss] \
                        else act_t[ss][2 * bias_slot + half]
                    q = act_s[t].index(ss)
                    nc.scalar.activation(
                        hts[t][(q % 2) * K:(q % 2) * K + K, q // 2, :],
                        ps1[half * K:(half + 1) * K, :], GELU,
                        bias=b1sb[half * K:(half + 1) * K, bias_slot, ss:ss + 1])

                for s in range(H):
                    for mc in range(fullc[s]):
                        ps1 = s1p.tile([P, PASS], F32, tag="ps1")
                        for d in range(DT):
                            nc.tensor.matmul(
                                ps1[:, :],
                                w1sb[s][:, d, mc * P:(mc + 1) * P],
                                cur[s][:, d, :],
                                start=(d == 0), stop=(d == DT - 1))
                        for half in range(2):
                            s1_act(s, half, ps1, mc)
                    if s == leftover:
                        mc = fullc[s]
                        ps1 = s1p.tile([P, PASS], F32, tag="ps1")
                        for d in range(DT):
                            nc.tensor.matmul(
                                ps1[:K, :],
                                w1sb[s][:, d, mc * P:mc * P + K],
                                cur[s][:, d, :],
                                start=(d == 0), stop=(d == DT - 1))
                        s1_act(s, 0, ps1, mc)
                    if s in pair_after:
                        sA, sB = pair_after[s]
                        ps1 = s1p.tile([P, PASS], F32, tag="ps1")
                        for d in range(DT):
                            cA = fullc[sA] * P
                            nc.tensor.matmul(
                                ps1[0:K, :],
                                w1sb[sA][:, d, cA:cA + K],
                                cur[sA][:, d, :],
                                start=(d == 0), stop=(d == DT - 1),
                                tile_position=(0, 0))
                            cB = fullc[sB] * P
                            nc.tensor.matmul(
                                ps1[K:2 * K, :],
                                w1sb[sB][:, d, cB:cB + K],
                                cur[sB][:, d, :],
                                start=(d == 0), stop=(d == DT - 1),
                                tile_position=(0, K))
                        s1_act(sA, 0, ps1, fullc[sA])
                        s1_act(sB, 1, ps1, 3)

                # stage 2: addT[d,n] = (gate*W2)^T @ hT, d-major bf16.
                # PSUM->SBUF casts alternate scalar/vector; one fat store
                # per target (per-o-pair for the final target of the run).
                ncopy = 0
                for i_t, t in enumerate(t_order):
                    ns = len(act_s[t])
                    osb = osp.tile([P, DT, PASS], BF16, tag="osb")
                    last_t = (p == NPASS - 1 and i_t == H - 1)
                    for oh in range(3):
                        oslc = osb[:, oh * 2:oh * 2 + 2, :]
                        if ns == 0:
                            nc.vector.memset(oslc, 0.0)
                        else:
                            ps2 = s2p.tile([P, 2, PASS], F32, tag="ps2")
                            for oi in range(2):
                                o = oh * 2 + oi
                                for j in range(s2tiles[t]):
                                    nc.tensor.matmul(
                                        ps2[:, oi, :],
                                        w2sb[t][:, j, o * P:(o + 1) * P],
                                        hts[t][:, j, :],
                                        start=(j == 0), stop=(j == s2tiles[t] - 1))
                            if ncopy % 2 == 0:
                                nc.scalar.activation(oslc, ps2[:], COPY)
                            else:
                                nc.vector.tensor_copy(oslc, ps2[:])
                            ncopy += 1
                        if has_b2 and ns:
                            for oi in range(2):
                                nc.vector.tensor_scalar_add(
                                    osb[:, oh * 2 + oi, :], osb[:, oh * 2 + oi, :],
                                    b2sb[:, oh * 2 + oi, t:t + 1])
                        if last_t:
                            [nc.sync, nc.scalar, nc.sync][oh].dma_start(
                                outd[t, p, :, oh * 2:oh * 2 + 2, :], oslc)
                    if not last_t:
                        eng = [nc.sync, nc.gpsimd, nc.sync, nc.gpsimd,
                               nc.scalar, nc.sync, nc.gpsimd][i_t]
                        eng.dma_start(outd[t, p], osb[:])
    nc.compile()
    return nc


def prepare(inputs):
    """Host prep: gate fold + bf16 cast + layout permutes.
    Returns (in_maps, build_key)."""
    x = np.asarray(inputs["x"], dtype=np.float32)
    M = np.asarray(inputs["M"], dtype=np.float32)
    W1 = np.asarray(inputs["W1"], dtype=np.float32)
    b1 = np.asarray(inputs["b1"], dtype=np.float32)
    W2 = np.asarray(inputs["W2"], dtype=np.float32)
    b2 = np.asarray(inputs["b2"], dtype=np.float32)

    eye = np.eye(H, dtype=bool)
    gate = np.where((np.abs(M) > THR) & (~eye), M, np.zeros_like(M)).astype(np.float32)
    has_b2 = bool(np.any(b2))
    act = gate != 0.0
    act_t = tuple(tuple(int(t) for t in range(H) if act[s, t]) for s in range(H))
    act_s = tuple(tuple(int(s) for s in range(H) if act[s, t]) for t in range(H))

    # W1 columns packed per source in act_t order -> [H, 128, 6, 384] bf16
    w1f = np.zeros((H, D, 3 * P), np.float32)
    b1f = np.zeros((H, 3 * P), np.float32)
    for s in range(H):
        for i, t in enumerate(act_t[s]):
            w1f[s, :, i * K:(i + 1) * K] = W1[s, t]
            b1f[s, i * K:(i + 1) * K] = b1[s, t]
    w1h = np.ascontiguousarray(
        w1f.reshape(H, DT, P, 3 * P).transpose(0, 2, 1, 3)).astype(ml_dtypes.bfloat16)
    # slot 3: odd-tail bias replicated to both partition halves
    b1x = np.zeros((H, 4, P), np.float32)
    b1x[:, :3, :] = b1f.reshape(H, 3, P)
    for s in range(H):
        ncols = len(act_t[s]) * K
        if ncols % P:
            c0 = (ncols // P) * P
            b1x[s, 3, 0:K] = b1f[s, c0:c0 + K]
            b1x[s, 3, K:2 * K] = b1f[s, c0:c0 + K]
    b1h = np.ascontiguousarray(b1x.transpose(2, 1, 0))

    # gate-scaled W2 rows packed per target in act_s order -> [H, 128, 3, 768]
    w2f = np.zeros((H, 3 * P, D), np.float32)
    for t in range(H):
        for q, s in enumerate(act_s[t]):
            w2f[t, q * K:(q + 1) * K, :] = gate[s, t] * W2[s, t]
    w2h = np.ascontiguousarray(
        w2f.reshape(H, 3, P, D).transpose(0, 2, 1, 3)).astype(ml_dtypes.bfloat16)

    # gate-folded b2 per target: [128, 6, 7]
    b2f = np.einsum("st,std->td", gate, b2).astype(np.float32)   # [H, D]
    b2h = np.ascontiguousarray(b2f.reshape(H, DT, P).transpose(2, 1, 0))

    in_maps = []
    for b in range(B):
        xbf = x[:, b].astype(ml_dtypes.bfloat16)
        # [s, q, p, o, n]: element = xbf[s, q*PASS+n, o*P+p]
        xtb = np.ascontiguousarray(
            xbf.reshape(H, NPASS, PASS, DT, P).transpose(0, 1, 4, 3, 2))
        in_maps.append({
            "xtd": xtb, "w1h": w1h, "w2h": w2h, "b1h": b1h, "b2h": b2h,
        })
    return in_maps, (has_b2, act_t, act_s)


def assemble(outs, x):
    """Per-core outd [H, NPASS, 128, 6, 512] bf16 (the ADD term) -> full
    [H, B, S, D] f32 output with the x residual applied host-side."""
    adds = []
    for b in range(B):
        o = np.asarray(outs[b]["outd"])
        # add[t, q*512+n, o*128+p] = outd[t, q, p, o, n]
        adds.append(o.transpose(0, 1, 4, 3, 2).reshape(H, S, D).astype(np.float32))
    add = np.stack(adds, axis=1)            # [H, B, S, D]
    return np.asarray(x, dtype=np.float32) + add


def kernel(**inputs):
    in_maps, key = prepare(inputs)
    runner = _get_runner(key)
    outs = runner.run(in_maps)
    return assemble(outs, inputs["x"])


class _Runner:
    """Cached PJRT executor for the SPMD bass kernel (8 cores, no donation)."""

    def __init__(self, nc):
        import jax
        from jax.sharding import Mesh, PartitionSpec, NamedSharding
        from jax.experimental.shard_map import shard_map
        from concourse import bass2jax
        bass2jax.install_neuronx_cc_hook()

        self.jax = jax
        part_name = nc.partition_id_tensor.name if nc.partition_id_tensor else None
        in_names, out_names, out_avals, zero_shapes = [], [], [], []
        for alloc in nc.m.functions[0].allocations:
            if not isinstance(alloc, mybir.MemoryLocationSet):
                continue
            name = alloc.memorylocations[0].name
            if alloc.kind == "ExternalInput":
                if name != part_name:
                    in_names.append(name)
            elif alloc.kind == "ExternalOutput":
                out_names.append(name)
                shape = tuple(alloc.tensor_shape)
                dtype = mybir.dt.np(alloc.dtype)
                out_avals.append(jax.core.ShapedArray(shape, dtype))
                zero_shapes.append((shape, dtype))
        self.n_params = len(in_names)
        self.in_names = list(in_names)
        self.out_names = out_names
        self.out_avals = out_avals
        self.zero_shapes = zero_shapes
        bind_names = tuple(in_names) + tuple(out_names)
        if part_name is not None:
            bind_names = bind_names + (part_name,)

        def _body(*args):
            operands = list(args)
            if part_name is not None:
                operands.append(bass2jax.partition_id_tensor())
            outs = bass2jax._bass_exec_p.bind(
                *operands,
                out_avals=tuple(out_avals),
                in_names=bind_names,
                out_names=tuple(out_names),
                lowering_input_output_aliases=(),
                sim_require_finite=True,
                sim_require_nnan=True,
                nc=nc,
            )
            return tuple(outs)

        devices = jax.devices()[:B]
        self.mesh = Mesh(np.asarray(devices), ("core",))
        spec = PartitionSpec("core")
        self.sharding = NamedSharding(self.mesh, spec)
        n_in = self.n_params + len(out_names)
        self.fn = jax.jit(
            shard_map(_body, mesh=self.mesh,
                      in_specs=(spec,) * n_in,
                      out_specs=(spec,) * len(out_names),
                      check_rep=False),
            keep_unused=True,
        )

    def _concat_args(self, in_maps):
        args = []
        for i, name in enumerate(self.in_names):
            args.append(np.concatenate([np.asarray(m[name]) for m in in_maps], axis=0))
        for shape, dtype in self.zero_shapes:
            args.append(np.zeros((B * shape[0],) + shape[1:], dtype))
        return args

    def run(self, in_maps):
        out_arrs = self.fn(*self._concat_args(in_maps))
        res = []
        for c in range(B):
            d = {}
            for i, name in enumerate(self.out_names):
                shape = self.out_avals[i].shape
                d[name] = np.asarray(out_arrs[i]).reshape((B,) + shape)[c]
            res.append(d)
        return res


def _build_from_key(key):
    has_b2, act_t, act_s = key
    return _build(has_b2=has_b2, act_t=act_t, act_s=act_s)


def _get_runner(key) -> _Runner:
    ck = ("runner", key)
    if ck not in _CACHE:
        _CACHE[ck] = _Runner(_build_from_key(key))
    return _CACHE[ck]


# revision 16
# speedup vs baseline: 1.0325x; 1.0325x over previous
"""CrossHazardInteractionLayer TRN2 kernel (v5).

Data-parallel over batch B=8 -> 8 NeuronCores (one batch element each).
Host prep: fold the |M|>thr gate into W2 (pre-scaled), cast W1/W2/x to
bf16, transpose x to feature-major (d on partitions) once.  Device:
  stage 1 per source s: hT[(t,k), n] = gelu(x[s]^T-tiles @ W1[s,:]) for
    all active targets, packed 2 targets per 128-col chunk; exact-erf
    GELU fused into the PSUM->SBUF copy on the scalar engine.
  stage 2 per target t: addT[d, n] = sum over (s,k) j-tiles of
    (gate*W2)^T-stationary @ hT-moving, written bf16 d-major.  The
    x residual is added on the HOST in assemble() (it is pure O(N)
    data movement; keeping it off-device halves the DVE drain load and
    lets the PSUM->SBUF cast split between scalar and vector engines).

Trace-driven scheduling rules (vs the 251us v2 baseline):
  - Each engine owns ~4 DMA-completion semaphore slots; a 5th doorbell
    BLOCKS the engine until an earlier DMA completes.  The scalar
    engine runs the latency-critical GELUs, so it rings only a 3-door
    startup prefix (b1 + 2 w2 tiles); everything else rides sync/gpsimd
    (whole-tile transfers, 0.6-0.8MB each for DMA efficiency).
  - w2 is prefetched during pass-0 stage 1 (baseline parked it behind
    x tiles on gpsimd -> 8us of PE stalls + a HAM re-throttle).
  - x for pass p+1 is prefetched at the TOP of pass p.
  - stage-2 partial k-tiles (targets with odd #sources) are padded to
    full 128 rows (w2 rows zero-padded host-side, h tail rows memset)
    so their LDWEIGHTS pipeline through the background weight buffer.
  - stage-2 targets run in h-availability order; PSUM->SBUF copies
    alternate scalar/vector; stores are one fat DMA per target except
    the very last target, which stores per-o-pair to shrink the tail.
"""

import numpy as np
import ml_dtypes

import concourse.bass as bass
import concourse.mybir as mybir
import concourse.tile as tile
from concourse import bacc

H = 7
B = 8
S = 2048
D = 768
K = 64
P = 128
PASS = 512          # seq cols per pass
NPASS = S // PASS
DT = D // P         # d-tiles (6)
THR = 0.05

F32 = mybir.dt.float32
BF16 = mybir.dt.bfloat16
GELU = mybir.ActivationFunctionType.Gelu
COPY = mybir.ActivationFunctionType.Copy

_CACHE: dict = {}


def _build(has_b2=False, act_t=None, act_s=None):
    """act_t[s] = tuple of active targets for source s (packing order);
    act_s[t] = tuple of active sources for target t (packing order)."""
    import math
    if act_t is None:
        act_t = tuple(tuple(t for t in range(H) if t != s) for s in range(H))
    if act_s is None:
        act_s = tuple(tuple(s for s in range(H) if s != t) for t in range(H))
    s2tiles = [math.ceil(len(act_s[t]) * K / P) for t in range(H)]
    # targets whose last k-tile is only half-filled (padded to 128 rows)
    s2pad = [t for t in range(H) if len(act_s[t]) and (len(act_s[t]) * K) % P]

    # odd-tail pairing for stage 1
    fullc = [len(act_t[s]) * K // P for s in range(H)]
    odd = [s for s in range(H) if (len(act_t[s]) * K) % P]
    pair_after = {}
    for i in range(0, len(odd) - 1, 2):
        pair_after[odd[i + 1]] = (odd[i], odd[i + 1])
    leftover = odd[-1] if len(odd) % 2 else None

    # stage-1 chunk emission index of each (s,t) GELU -> stage-2 target order
    emit_idx = {}
    ctr = 0
    for s in range(H):
        for mc in range(fullc[s]):
            emit_idx[(s, act_t[s][2 * mc])] = ctr
            emit_idx[(s, act_t[s][2 * mc + 1])] = ctr
            ctr += 1
        if s == leftover:
            emit_idx[(s, act_t[s][2 * fullc[s]])] = ctr
            ctr += 1
        if s in pair_after:
            sA, sB = pair_after[s]
            emit_idx[(sA, act_t[sA][2 * fullc[sA]])] = ctr
            emit_idx[(sB, act_t[sB][2 * fullc[sB]])] = ctr
            ctr += 1
    t_last = {t: max([emit_idx[(s, t)] for s in act_s[t]] or [-1])
              for t in range(H)}
    t_order = sorted(range(H), key=lambda t: (t_last[t], t))

    nc = bacc.Bacc("TRN2", target_bir_lowering=False, debug=False)
    xtd = nc.declare_dram_parameter("xtd", [H, NPASS, P, DT, PASS], BF16, isOutput=False)
    w1h = nc.declare_dram_parameter("w1h", [H, P, DT, 3 * P], BF16, isOutput=False)
    w2h = nc.declare_dram_parameter("w2h", [H, P, 3, D], BF16, isOutput=False)
    b1h = nc.declare_dram_parameter("b1h", [P, 4, H], F32, isOutput=False)
    b2h = nc.declare_dram_parameter("b2h", [P, DT, H], F32, isOutput=False)
    outd = nc.declare_dram_parameter("outd", [H, NPASS, P, DT, PASS], BF16, isOutput=True)

    with tile.TileContext(nc) as tc:
        with tc.tile_pool(name="static", bufs=1) as st, \
             tc.tile_pool(name="xt", bufs=2) as xtp, \
             tc.tile_pool(name="ht", bufs=2) as htp, \
             tc.tile_pool(name="osb_s", bufs=3) as osp_s, \
             tc.tile_pool(name="osb_v", bufs=3) as osp_v, \
             tc.tile_pool(name="s1_ps", bufs=4, space="PSUM") as s1p, \
             tc.tile_pool(name="s2_ps", bufs=2, space="PSUM") as s2p:

            b1sb = st.tile([P, 4, H], F32, tag="b1sb")
            nc.scalar.dma_start(b1sb[:], b1h[:])
            w1sb = [None] * H
            w2sb = [None] * H
            b2sb = None

            xts = [[None] * H for _ in range(2)]

            def load_x(p):
                dst = xts[p % 2]
                for s in range(H):
                    xt = xtp.tile([P, DT, PASS], BF16, tag=f"xt{s}")
                    eng = [nc.sync, nc.gpsimd][(s + p) % 2]
                    eng.dma_start(xt[:], xtd[s, p])
                    dst[s] = xt

            # ---- passes over the sequence ----
            for p in range(NPASS):
                if p == 0:
                    # startup: whole-tile w1/x alternating sync/gpsimd in
                    # consumption order; first 2 w2 tiles on scalar's free
                    # startup slots, the rest behind the pass-0 tiles
                    for s in range(H):
                        w = st.tile([P, DT, 3 * P], BF16, tag=f"w1_{s}",
                                    name=f"w1_{s}")
                        [nc.gpsimd, nc.sync][s % 2].dma_start(w[:], w1h[s])
                        w1sb[s] = w
                        xt = xtp.tile([P, DT, PASS], BF16, tag=f"xt{s}")
                        [nc.sync, nc.gpsimd][s % 2].dma_start(xt[:], xtd[s, 0])
                        xts[0][s] = xt
                    if has_b2:
                        b2sb = st.tile([P, DT, H], F32, tag="b2sb")
                        nc.gpsimd.dma_start(b2sb[:], b2h[:])
                    nw2 = 0
                    for t in t_order:
                        if not act_s[t]:
                            continue
                        w = st.tile([P, 3, D], BF16, tag=f"w2_{t}", name=f"w2_{t}")
                        eng = nc.scalar if nw2 < 2 else \
                            [nc.sync, nc.gpsimd][nw2 % 2]
                        eng.dma_start(w[:], w2h[t])
                        w2sb[t] = w
                        nw2 += 1
                if p + 1 < NPASS:
                    load_x(p + 1)
                cur = xts[p % 2]

                hts = [htp.tile([P, 3, PASS], BF16, tag=f"ht{t}", name=f"ht{t}")
                       for t in range(H)]
                # zero the never-written tail rows of half-filled k-tiles so
                # the padded full-row stage-2 matmuls multiply 0-weights by
                # finite values
                for t in s2pad:
                    r0 = (len(act_s[t]) * K) % P
                    nc.vector.memset(hts[t][r0:, s2tiles[t] - 1, :], 0.0)

                # stage 1
                def s1_act(ss, half, ps1, bias_slot):
                    t = act_t[ss][2 * fullc[ss]] if bias_slot >= fullc[ss] \
                        else act_t[ss][2 * bias_slot + half]
                    q = act_s[t].index(ss)
                    nc.scalar.activation(
                        hts[t][(q % 2) * K:(q % 2) * K + K, q // 2, :],
                        ps1[half * K:(half + 1) * K, :], GELU,
                        bias=b1sb[half * K:(half + 1) * K, bias_slot, ss:ss + 1])

                for s in range(H):
                    for mc in range(fullc[s]):
                        ps1 = s1p.tile([P, PASS], F32, tag="ps1")
                        for d in range(DT):
                            nc.tensor.matmul(
                                ps1[:, :],
                                w1sb[s][:, d, mc * P:(mc + 1) * P],
                                cur[s][:, d, :],
                                start=(d == 0), stop=(d == DT - 1))
                        for half in range(2):
                            s1_act(s, half, ps1, mc)
                    if s == leftover:
                        mc = fullc[s]
                        ps1 = s1p.tile([P, PASS], F32, tag="ps1")
                        for d in range(DT):
                            nc.tensor.matmul(
                                ps1[:K, :],
                                w1sb[s][:, d, mc * P:mc * P + K],
                                cur[s][:, d, :],
                                start=(d == 0), stop=(d == DT - 1))
                        s1_act(s, 0, ps1, mc)
                    if s in pair_after:
                        sA, sB = pair_after[s]
                        ps1 = s1p.tile([P, PASS], F32, tag="ps1")
                        for d in range(DT):
                            cA = fullc[sA] * P
                            nc.tensor.matmul(
                                ps1[0:K, :],
                                w1sb[sA][:, d, cA:cA + K],
                                cur[sA][:, d, :],
                                start=(d == 0), stop=(d == DT - 1),
                                tile_position=(0, 0))
                            cB = fullc[sB] * P
                            nc.tensor.matmul(
                                ps1[K:2 * K, :],
                                w1sb[sB][:, d, cB:cB + K],
                                cur[sB][:, d, :],
                                start=(d == 0), stop=(d == DT - 1),
                                tile_position=(0, K))
                        s1_act(sA, 0, ps1, fullc[sA])
                        s1_act(sB, 1, ps1, 3)

                # stage 2: addT[d,n] = (gate*W2)^T @ hT, d-major bf16.
                # PSUM->SBUF casts alternate scalar/vector into SEPARATE
                # per-engine tile pools (Tile orders same-tile writers
                # cross-engine at tile granularity, so sharing one tile
                # between the two copy engines chains them); stores are
                # per-o-pair on sync/gpsimd (scalar rings no stage-2 DMA,
                # except the very tail of the run).
                ncopy = 0
                for i_t, t in enumerate(t_order):
                    ns = len(act_s[t])
                    last_t = (p == NPASS - 1 and i_t == H - 1)
                    for oh in range(3):
                        use_s = (ncopy % 2 == 0)
                        osb = (osp_s if use_s else osp_v).tile(
                            [P, 2, PASS], BF16, tag="osbs" if use_s else "osbv")
                        if ns == 0:
                            nc.vector.memset(osb[:], 0.0)
                        else:
                            ps2 = s2p.tile([P, 2, PASS], F32, tag="ps2")
                            for oi in range(2):
                                o = oh * 2 + oi
                                for j in range(s2tiles[t]):
                                    nc.tensor.matmul(
                                        ps2[:, oi, :],
                                        w2sb[t][:, j, o * P:(o + 1) * P],
                                        hts[t][:, j, :],
                                        start=(j == 0), stop=(j == s2tiles[t] - 1))
                            if use_s:
                                nc.scalar.activation(osb[:], ps2[:], COPY)
                            else:
                                nc.vector.tensor_copy(osb[:], ps2[:])
                        ncopy += 1
                        if has_b2 and ns:
                            for oi in range(2):
                                nc.vector.tensor_scalar_add(
                                    osb[:, oi, :], osb[:, oi, :],
                                    b2sb[:, oh * 2 + oi, t:t + 1])
                        if last_t:
                            eng = [nc.sync, nc.scalar, nc.sync][oh]
                        else:
                            eng = [nc.sync, nc.gpsimd][ncopy % 2]
                        eng.dma_start(outd[t, p, :, oh * 2:oh * 2 + 2, :], osb[:])
    nc.compile()
    return nc


def prepare(inputs):
    """Host prep: gate fold + bf16 cast + layout permutes.
    Returns (in_maps, build_key)."""
    x = np.asarray(inputs["x"], dtype=np.float32)
    M = np.asarray(inputs["M"], dtype=np.float32)
    W1 = np.asarray(inputs["W1"], dtype=np.float32)
    b1 = np.asarray(inputs["b1"], dtype=np.float32)
    W2 = np.asarray(inputs["W2"], dtype=np.float32)
    b2 = np.asarray(inputs["b2"], dtype=np.float32)

    eye = np.eye(H, dtype=bool)
    gate = np.where((np.abs(M) > THR) & (~eye), M, np.zeros_like(M)).astype(np.float32)
    has_b2 = bool(np.any(b2))
    act = gate != 0.0
    act_t = tuple(tuple(int(t) for t in range(H) if act[s, t]) for s in range(H))
    act_s = tuple(tuple(int(s) for s in range(H) if act[s, t]) for t in range(H))

    # W1 columns packed per source in act_t order -> [H, 128, 6, 384] bf16
    w1f = np.zeros((H, D, 3 * P), np.float32)
    b1f = np.zeros((H, 3 * P), np.float32)
    for s in range(H):
        for i, t in enumerate(act_t[s]):
            w1f[s, :, i * K:(i + 1) * K] = W1[s, t]
            b1f[s, i * K:(i + 1) * K] = b1[s, t]
    w1h = np.ascontiguousarray(
        w1f.reshape(H, DT, P, 3 * P).transpose(0, 2, 1, 3)).astype(ml_dtypes.bfloat16)
    # slot 3: odd-tail bias replicated to both partition halves
    b1x = np.zeros((H, 4, P), np.float32)
    b1x[:, :3, :] = b1f.reshape(H, 3, P)
    for s in range(H):
        ncols = len(act_t[s]) * K
        if ncols % P:
            c0 = (ncols // P) * P
            b1x[s, 3, 0:K] = b1f[s, c0:c0 + K]
            b1x[s, 3, K:2 * K] = b1f[s, c0:c0 + K]
    b1h = np.ascontiguousarray(b1x.transpose(2, 1, 0))

    # gate-scaled W2 rows packed per target in act_s order -> [H, 128, 3, 768]
    w2f = np.zeros((H, 3 * P, D), np.float32)
    for t in range(H):
        for q, s in enumerate(act_s[t]):
            w2f[t, q * K:(q + 1) * K, :] = gate[s, t] * W2[s, t]
    w2h = np.ascontiguousarray(
        w2f.reshape(H, 3, P, D).transpose(0, 2, 1, 3)).astype(ml_dtypes.bfloat16)

    # gate-folded b2 per target: [128, 6, 7]
    b2f = np.einsum("st,std->td", gate, b2).astype(np.float32)   # [H, D]
    b2h = np.ascontiguousarray(b2f.reshape(H, DT, P).transpose(2, 1, 0))

    in_maps = []
    for b in range(B):
        xbf = x[:, b].astype(ml_dtypes.bfloat16)
        # [s, q, p, o, n]: element = xbf[s, q*PASS+n, o*P+p]
        xtb = np.ascontiguousarray(
            xbf.reshape(H, NPASS, PASS, DT, P).transpose(0, 1, 4, 3, 2))
        in_maps.append({
            "xtd": xtb, "w1h": w1h, "w2h": w2h, "b1h": b1h, "b2h": b2h,
        })
    return in_maps, (has_b2, act_t, act_s)


def assemble(outs, x):
    """Per-core outd [H, NPASS, 128, 6, 512] bf16 (the ADD term) -> full
    [H, B, S, D] f32 output with the x residual applied host-side."""
    adds = []
    for b in range(B):
        o = np.asarray(outs[b]["outd"])
        # add[t, q*512+n, o*128+p] = outd[t, q, p, o, n]
        adds.append(o.transpose(0, 1, 4, 3, 2).reshape(H, S, D).astype(np.float32))
    add = np.stack(adds, axis=1)            # [H, B, S, D]
    return np.asarray(x, dtype=np.float32) + add


def kernel(**inputs):
    in_maps, key = prepare(inputs)
    runner = _get_runner(key)
    outs = runner.run(in_maps)
    return assemble(outs, inputs["x"])


class _Runner:
    """Cached PJRT executor for the SPMD bass kernel (8 cores, no donation)."""

    def __init__(self, nc):
        import jax
        from jax.sharding import Mesh, PartitionSpec, NamedSharding
        from jax.experimental.shard_map import shard_map
        from concourse import bass2jax
        bass2jax.install_neuronx_cc_hook()

        self.jax = jax
        part_name = nc.partition_id_tensor.name if nc.partition_id_tensor else None
        in_names, out_names, out_avals, zero_shapes = [], [], [], []
        for alloc in nc.m.functions[0].allocations:
            if not isinstance(alloc, mybir.MemoryLocationSet):
                continue
            name = alloc.memorylocations[0].name
            if alloc.kind == "ExternalInput":
                if name != part_name:
                    in_names.append(name)
            elif alloc.kind == "ExternalOutput":
                out_names.append(name)
                shape = tuple(alloc.tensor_shape)
                dtype = mybir.dt.np(alloc.dtype)
                out_avals.append(jax.core.ShapedArray(shape, dtype))
                zero_shapes.append((shape, dtype))
        self.n_params = len(in_names)
        self.in_names = list(in_names)
        self.out_names = out_names
        self.out_avals = out_avals
        self.zero_shapes = zero_shapes
        bind_names = tuple(in_names) + tuple(out_names)
        if part_name is not None:
            bind_names = bind_names + (part_name,)

        def _body(*args):
            operands = list(args)
            if part_name is not None:
                operands.append(bass2jax.partition_id_tensor())
            outs = bass2jax._bass_exec_p.bind(
                *operands,
                out_avals=tuple(out_avals),
                in_names=bind_names,
                out_names=tuple(out_names),
                lowering_input_output_aliases=(),
                sim_require_finite=True,
                sim_require_nnan=True,
                nc=nc,
            )
            return tuple(outs)

        devices = jax.devices()[:B]
        self.mesh = Mesh(np.asarray(devices), ("core",))
        spec = PartitionSpec("core")
        self.sharding = NamedSharding(self.mesh, spec)
        n_in = self.n_params + len(out_names)
        self.fn = jax.jit(
            shard_map(_body, mesh=self.mesh,
                      in_specs=(spec,) * n_in,
                      out_specs=(spec,) * len(out_names),
                      check_rep=False),
            keep_unused=True,
        )

    def _concat_args(self, in_maps):
        args = []
        for i, name in enumerate(self.in_names):
            args.append(np.concatenate([np.asarray(m[name]) for m in in_maps], axis=0))
        for shape, dtype in self.zero_shapes:
            args.append(np.zeros((B * shape[0],) + shape[1:], dtype))
        return args

    def run(self, in_maps):
        out_arrs = self.fn(*self._concat_args(in_maps))
        res = []
        for c in range(B):
            d = {}
            for i, name in enumerate(self.out_names):
                shape = self.out_avals[i].shape
                d[name] = np.asarray(out_arrs[i]).reshape((B,) + shape)[c]
            res.append(d)
        return res


def _build_from_key(key):
    has_b2, act_t, act_s = key
    return _build(has_b2=has_b2, act_t=act_t, act_s=act_s)


def _get_runner(key) -> _Runner:
    ck = ("runner", key)
    if ck not in _CACHE:
        _CACHE[ck] = _Runner(_build_from_key(key))
    return _CACHE[ck]


# revision 17
# speedup vs baseline: 1.0811x; 1.0471x over previous
"""CrossHazardInteractionLayer TRN2 kernel (v5).

Data-parallel over batch B=8 -> 8 NeuronCores (one batch element each).
Host prep: fold the |M|>thr gate into W2 (pre-scaled), cast W1/W2/x to
bf16, transpose x to feature-major (d on partitions) once.  Device:
  stage 1 per source s: hT[(t,k), n] = gelu(x[s]^T-tiles @ W1[s,:]) for
    all active targets, packed 2 targets per 128-col chunk; exact-erf
    GELU fused into the PSUM->SBUF copy on the scalar engine.
  stage 2 per target t: addT[d, n] = sum over (s,k) j-tiles of
    (gate*W2)^T-stationary @ hT-moving, written bf16 d-major.  The
    x residual is added on the HOST in assemble() (it is pure O(N)
    data movement; keeping it off-device halves the DVE drain load and
    lets the PSUM->SBUF cast split between scalar and vector engines).

Trace-driven scheduling rules (vs the 251us v2 baseline):
  - Each engine owns ~4 DMA-completion semaphore slots; a 5th doorbell
    BLOCKS the engine until an earlier DMA completes.  The scalar
    engine runs the latency-critical GELUs, so it rings only a 3-door
    startup prefix (b1 + 2 w2 tiles); everything else rides sync/gpsimd
    (whole-tile transfers, 0.6-0.8MB each for DMA efficiency).
  - w2 is prefetched during pass-0 stage 1 (baseline parked it behind
    x tiles on gpsimd -> 8us of PE stalls + a HAM re-throttle).
  - x for pass p+1 is prefetched at the TOP of pass p.
  - stage-2 partial k-tiles (targets with odd #sources) are padded to
    full 128 rows (w2 rows zero-padded host-side, h tail rows memset)
    so their LDWEIGHTS pipeline through the background weight buffer.
  - stage-2 targets run in h-availability order; PSUM->SBUF copies
    alternate scalar/vector; stores are one fat DMA per target except
    the very last target, which stores per-o-pair to shrink the tail.
"""

import numpy as np
import ml_dtypes

import concourse.bass as bass
import concourse.mybir as mybir
import concourse.tile as tile
from concourse import bacc

H = 7
B = 8
S = 2048
D = 768
K = 64
P = 128
PASS = 512          # seq cols per pass
NPASS = S // PASS
DT = D // P         # d-tiles (6)
THR = 0.05

F32 = mybir.dt.float32
BF16 = mybir.dt.bfloat16
GELU = mybir.ActivationFunctionType.Gelu
COPY = mybir.ActivationFunctionType.Copy

_CACHE: dict = {}


def _build(has_b2=False, act_t=None, act_s=None):
    """act_t[s] = tuple of active targets for source s (packing order);
    act_s[t] = tuple of active sources for target t (packing order)."""
    import math
    if act_t is None:
        act_t = tuple(tuple(t for t in range(H) if t != s) for s in range(H))
    if act_s is None:
        act_s = tuple(tuple(s for s in range(H) if s != t) for t in range(H))
    s2tiles = [math.ceil(len(act_s[t]) * K / P) for t in range(H)]
    # targets whose last k-tile is only half-filled (padded to 128 rows)
    s2pad = [t for t in range(H) if len(act_s[t]) and (len(act_s[t]) * K) % P]

    # odd-tail pairing for stage 1
    fullc = [len(act_t[s]) * K // P for s in range(H)]
    odd = [s for s in range(H) if (len(act_t[s]) * K) % P]
    pair_after = {}
    for i in range(0, len(odd) - 1, 2):
        pair_after[odd[i + 1]] = (odd[i], odd[i + 1])
    leftover = odd[-1] if len(odd) % 2 else None

    # stage-1 chunk emission index of each (s,t) GELU -> stage-2 target order
    emit_idx = {}
    ctr = 0
    for s in range(H):
        for mc in range(fullc[s]):
            emit_idx[(s, act_t[s][2 * mc])] = ctr
            emit_idx[(s, act_t[s][2 * mc + 1])] = ctr
            ctr += 1
        if s == leftover:
            emit_idx[(s, act_t[s][2 * fullc[s]])] = ctr
            ctr += 1
        if s in pair_after:
            sA, sB = pair_after[s]
            emit_idx[(sA, act_t[sA][2 * fullc[sA]])] = ctr
            emit_idx[(sB, act_t[sB][2 * fullc[sB]])] = ctr
            ctr += 1
    t_last = {t: max([emit_idx[(s, t)] for s in act_s[t]] or [-1])
              for t in range(H)}
    t_order = sorted(range(H), key=lambda t: (t_last[t], t))

    nc = bacc.Bacc("TRN2", target_bir_lowering=False, debug=False)
    xtd = nc.declare_dram_parameter("xtd", [H, NPASS, P, DT, PASS], BF16, isOutput=False)
    w1h = nc.declare_dram_parameter("w1h", [H, P, DT, 3 * P], BF16, isOutput=False)
    w2h = nc.declare_dram_parameter("w2h", [H, P, 3, D], BF16, isOutput=False)
    b1h = nc.declare_dram_parameter("b1h", [P, 4, H], F32, isOutput=False)
    b2h = nc.declare_dram_parameter("b2h", [P, DT, H], F32, isOutput=False)
    outd = nc.declare_dram_parameter("outd", [H, NPASS, P, DT, PASS], BF16, isOutput=True)

    with tile.TileContext(nc) as tc:
        with tc.tile_pool(name="static", bufs=1) as st, \
             tc.tile_pool(name="xt", bufs=2) as xtp, \
             tc.tile_pool(name="ht", bufs=2) as htp, \
             tc.tile_pool(name="osb_s", bufs=3) as osp_s, \
             tc.tile_pool(name="osb_v", bufs=3) as osp_v, \
             tc.tile_pool(name="s1_ps", bufs=4, space="PSUM") as s1p, \
             tc.tile_pool(name="s2_ps", bufs=2, space="PSUM") as s2p:

            b1sb = st.tile([P, 4, H], F32, tag="b1sb")
            nc.scalar.dma_start(b1sb[:], b1h[:])
            w1sb = [None] * H
            w2sb = [None] * H
            b2sb = None

            xts = [[None] * H for _ in range(2)]
            hts2 = [[None] * H for _ in range(2)]
            state = {"ncopy": 0}

            def s1_act(ss, half, ps1, bias_slot, hts):
                t = act_t[ss][2 * fullc[ss]] if bias_slot >= fullc[ss] \
                    else act_t[ss][2 * bias_slot + half]
                q = act_s[t].index(ss)
                nc.scalar.activation(
                    hts[t][(q % 2) * K:(q % 2) * K + K, q // 2, :],
                    ps1[half * K:(half + 1) * K, :], GELU,
                    bias=b1sb[half * K:(half + 1) * K, bias_slot, ss:ss + 1])

            def s1_chunks(p):
                """Stage-1 of pass p as a list of chunk-emitter thunks."""
                cur = xts[p % 2]
                hts = hts2[p % 2]
                out = []

                def full(s, mc):
                    def f():
                        ps1 = s1p.tile([P, PASS], F32, tag="ps1")
                        for d in range(DT):
                            nc.tensor.matmul(
                                ps1[:, :],
                                w1sb[s][:, d, mc * P:(mc + 1) * P],
                                cur[s][:, d, :],
                                start=(d == 0), stop=(d == DT - 1))
                        for half in range(2):
                            s1_act(s, half, ps1, mc, hts)
                    return f

                def tail(s):
                    def f():
                        mc = fullc[s]
                        ps1 = s1p.tile([P, PASS], F32, tag="ps1")
                        for d in range(DT):
                            nc.tensor.matmul(
                                ps1[:K, :],
                                w1sb[s][:, d, mc * P:mc * P + K],
                                cur[s][:, d, :],
                                start=(d == 0), stop=(d == DT - 1))
                        s1_act(s, 0, ps1, mc, hts)
                    return f

                def pair(sA, sB):
                    def f():
                        ps1 = s1p.tile([P, PASS], F32, tag="ps1")
                        for d in range(DT):
                            cA = fullc[sA] * P
                            nc.tensor.matmul(
                                ps1[0:K, :],
                                w1sb[sA][:, d, cA:cA + K],
                                cur[sA][:, d, :],
                                start=(d == 0), stop=(d == DT - 1),
                                tile_position=(0, 0))
                            cB = fullc[sB] * P
                            nc.tensor.matmul(
                                ps1[K:2 * K, :],
                                w1sb[sB][:, d, cB:cB + K],
                                cur[sB][:, d, :],
                                start=(d == 0), stop=(d == DT - 1),
                                tile_position=(0, K))
                        s1_act(sA, 0, ps1, fullc[sA], hts)
                        s1_act(sB, 1, ps1, 3, hts)
                    return f

                for s in range(H):
                    for mc in range(fullc[s]):
                        out.append(full(s, mc))
                    if s == leftover:
                        out.append(tail(s))
                    if s in pair_after:
                        out.append(pair(*pair_after[s]))
                return out

            def s2_target(p, t, i_t):
                """Stage-2 of pass p for one target: 3 o-pair blocks of
                6 MMs + PSUM->SBUF cast (alternating scalar/vector pools)
                + per-o-pair store on sync/gpsimd."""
                hts = hts2[p % 2]
                ns = len(act_s[t])
                last_t = (p == NPASS - 1 and i_t == H - 1)
                for oh in range(3):
                    nco = state["ncopy"]
                    use_s = (nco % 2 == 0)
                    osb = (osp_s if use_s else osp_v).tile(
                        [P, 2, PASS], BF16, tag="osbs" if use_s else "osbv")
                    if ns == 0:
                        nc.vector.memset(osb[:], 0.0)
                    else:
                        ps2 = s2p.tile([P, 2, PASS], F32, tag="ps2")
                        for oi in range(2):
                            o = oh * 2 + oi
                            for j in range(s2tiles[t]):
                                nc.tensor.matmul(
                                    ps2[:, oi, :],
                                    w2sb[t][:, j, o * P:(o + 1) * P],
                                    hts[t][:, j, :],
                                    start=(j == 0), stop=(j == s2tiles[t] - 1))
                        if use_s:
                            nc.scalar.activation(osb[:], ps2[:], COPY)
                        else:
                            nc.vector.tensor_copy(osb[:], ps2[:])
                    state["ncopy"] = nco + 1
                    if has_b2 and ns:
                        for oi in range(2):
                            nc.vector.tensor_scalar_add(
                                osb[:, oi, :], osb[:, oi, :],
                                b2sb[:, oh * 2 + oi, t:t + 1])
                    if last_t:
                        eng = [nc.sync, nc.scalar, nc.sync][oh]
                    else:
                        eng = [nc.sync, nc.gpsimd][state["ncopy"] % 2]
                    eng.dma_start(outd[t, p, :, oh * 2:oh * 2 + 2, :], osb[:])

            # ---- software-pipelined passes: stage-1 of pass p interleaves
            # with stage-2 of pass p-1 (whose GELUs are a full pass old, so
            # its matmuls never wait on the scalar engine), the x prefetch
            # for p+1, and (late pass 0) the w2 loads.  Stores spread evenly
            # through the pass instead of bursting in a stage-2 window.
            for p in range(NPASS):
                if p == 0:
                    # startup: whole-tile w1/x alternating sync/gpsimd in
                    # consumption order; scalar rings only b1 (its queue
                    # must not delay the GELU stream)
                    for s in range(H):
                        w = st.tile([P, DT, 3 * P], BF16, tag=f"w1_{s}",
                                    name=f"w1_{s}")
                        [nc.gpsimd, nc.sync][s % 2].dma_start(w[:], w1h[s])
                        w1sb[s] = w
                        xt = xtp.tile([P, DT, PASS], BF16, tag=f"xt{s}")
                        [nc.sync, nc.gpsimd][s % 2].dma_start(xt[:], xtd[s, 0])
                        xts[0][s] = xt
                    if has_b2:
                        b2sb = st.tile([P, DT, H], F32, tag="b2sb")
                        nc.gpsimd.dma_start(b2sb[:], b2h[:])
                    for t in range(H):
                        if act_s[t]:
                            w2sb[t] = st.tile([P, 3, D], BF16, tag=f"w2_{t}",
                                              name=f"w2_{t}")

                hts2[p % 2] = [htp.tile([P, 3, PASS], BF16, tag=f"ht{t}",
                                        name=f"ht{t}") for t in range(H)]
                for t in s2pad:
                    r0 = (len(act_s[t]) * K) % P
                    nc.vector.memset(
                        hts2[p % 2][t][r0:, s2tiles[t] - 1, :], 0.0)

                chunks = s1_chunks(p)
                ncks = len(chunks)
                # interleave triggers: after chunk c, maybe ring a DMA and
                # maybe emit one stage-2 target of the previous pass
                x_ring = {}
                if p + 1 < NPASS:
                    for s in range(H):
                        c = (s + 1) * ncks // (H + 2)
                        eng = [nc.sync, nc.scalar, nc.gpsimd][s % 3]
                        x_ring.setdefault(c, []).append((s, eng))
                w2_ring = {}
                if p == 0:
                    for i, t in enumerate(t_order):
                        if not act_s[t]:
                            continue
                        c = ncks // 2 + i * (ncks // 2) // H
                        w2_ring.setdefault(c, []).append(t)
                s2_trig = {}
                if p > 0:
                    for i, t in enumerate(t_order):
                        c = (i + 1) * ncks // (H + 1)
                        s2_trig.setdefault(c, []).append((i, t))

                for c, chunk in enumerate(chunks):
                    chunk()
                    for s, eng in x_ring.get(c, ()):
                        xt = xtp.tile([P, DT, PASS], BF16, tag=f"xt{s}")
                        eng.dma_start(xt[:], xtd[s, p + 1])
                        xts[(p + 1) % 2][s] = xt
                    for t in w2_ring.get(c, ()):
                        [nc.sync, nc.gpsimd][t % 2].dma_start(
                            w2sb[t][:], w2h[t])
                    for i, t in s2_trig.get(c, ()):
                        s2_target(p - 1, t, i)

            # epilogue: stage-2 of the final pass
            for i, t in enumerate(t_order):
                s2_target(NPASS - 1, t, i)
    nc.compile()
    return nc


def prepare(inputs):
    """Host prep: gate fold + bf16 cast + layout permutes.
    Returns (in_maps, build_key)."""
    x = np.asarray(inputs["x"], dtype=np.float32)
    M = np.asarray(inputs["M"], dtype=np.float32)
    W1 = np.asarray(inputs["W1"], dtype=np.float32)
    b1 = np.asarray(inputs["b1"], dtype=np.float32)
    W2 = np.asarray(inputs["W2"], dtype=np.float32)
    b2 = np.asarray(inputs["b2"], dtype=np.float32)

    eye = np.eye(H, dtype=bool)
    gate = np.where((np.abs(M) > THR) & (~eye), M, np.zeros_like(M)).astype(np.float32)
    has_b2 = bool(np.any(b2))
    act = gate != 0.0
    act_t = tuple(tuple(int(t) for t in range(H) if act[s, t]) for s in range(H))
    act_s = tuple(tuple(int(s) for s in range(H) if act[s, t]) for t in range(H))

    # W1 columns packed per source in act_t order -> [H, 128, 6, 384] bf16
    w1f = np.zeros((H, D, 3 * P), np.float32)
    b1f = np.zeros((H, 3 * P), np.float32)
    for s in range(H):
        for i, t in enumerate(act_t[s]):
            w1f[s, :, i * K:(i + 1) * K] = W1[s, t]
            b1f[s, i * K:(i + 1) * K] = b1[s, t]
    w1h = np.ascontiguousarray(
        w1f.reshape(H, DT, P, 3 * P).transpose(0, 2, 1, 3)).astype(ml_dtypes.bfloat16)
    # slot 3: odd-tail bias replicated to both partition halves
    b1x = np.zeros((H, 4, P), np.float32)
    b1x[:, :3, :] = b1f.reshape(H, 3, P)
    for s in range(H):
        ncols = len(act_t[s]) * K
        if ncols % P:
            c0 = (ncols // P) * P
            b1x[s, 3, 0:K] = b1f[s, c0:c0 + K]
            b1x[s, 3, K:2 * K] = b1f[s, c0:c0 + K]
    b1h = np.ascontiguousarray(b1x.transpose(2, 1, 0))

    # gate-scaled W2 rows packed per target in act_s order -> [H, 128, 3, 768]
    w2f = np.zeros((H, 3 * P, D), np.float32)
    for t in range(H):
        for q, s in enumerate(act_s[t]):
            w2f[t, q * K:(q + 1) * K, :] = gate[s, t] * W2[s, t]
    w2h = np.ascontiguousarray(
        w2f.reshape(H, 3, P, D).transpose(0, 2, 1, 3)).astype(ml_dtypes.bfloat16)

    # gate-folded b2 per target: [128, 6, 7]
    b2f = np.einsum("st,std->td", gate, b2).astype(np.float32)   # [H, D]
    b2h = np.ascontiguousarray(b2f.reshape(H, DT, P).transpose(2, 1, 0))

    in_maps = []
    for b in range(B):
        xbf = x[:, b].astype(ml_dtypes.bfloat16)
        # [s, q, p, o, n]: element = xbf[s, q*PASS+n, o*P+p]
        xtb = np.ascontiguousarray(
            xbf.reshape(H, NPASS, PASS, DT, P).transpose(0, 1, 4, 3, 2))
        in_maps.append({
            "xtd": xtb, "w1h": w1h, "w2h": w2h, "b1h": b1h, "b2h": b2h,
        })
    return in_maps, (has_b2, act_t, act_s)


def assemble(outs, x):
    """Per-core outd [H, NPASS, 128, 6, 512] bf16 (the ADD term) -> full
    [H, B, S, D] f32 output with the x residual applied host-side."""
    adds = []
    for b in range(B):
        o = np.asarray(outs[b]["outd"])
        # add[t, q*512+n, o*128+p] = outd[t, q, p, o, n]
        adds.append(o.transpose(0, 1, 4, 3, 2).reshape(H, S, D).astype(np.float32))
    add = np.stack(adds, axis=1)            # [H, B, S, D]
    return np.asarray(x, dtype=np.float32) + add


def kernel(**inputs):
    in_maps, key = prepare(inputs)
    runner = _get_runner(key)
    outs = runner.run(in_maps)
    return assemble(outs, inputs["x"])


class _Runner:
    """Cached PJRT executor for the SPMD bass kernel (8 cores, no donation)."""

    def __init__(self, nc):
        import jax
        from jax.sharding import Mesh, PartitionSpec, NamedSharding
        from jax.experimental.shard_map import shard_map
        from concourse import bass2jax
        bass2jax.install_neuronx_cc_hook()

        self.jax = jax
        part_name = nc.partition_id_tensor.name if nc.partition_id_tensor else None
        in_names, out_names, out_avals, zero_shapes = [], [], [], []
        for alloc in nc.m.functions[0].allocations:
            if not isinstance(alloc, mybir.MemoryLocationSet):
                continue
            name = alloc.memorylocations[0].name
            if alloc.kind == "ExternalInput":
                if name != part_name:
                    in_names.append(name)
            elif alloc.kind == "ExternalOutput":
                out_names.append(name)
                shape = tuple(alloc.tensor_shape)
                dtype = mybir.dt.np(alloc.dtype)
                out_avals.append(jax.core.ShapedArray(shape, dtype))
                zero_shapes.append((shape, dtype))
        self.n_params = len(in_names)
        self.in_names = list(in_names)
        self.out_names = out_names
        self.out_avals = out_avals
        self.zero_shapes = zero_shapes
        bind_names = tuple(in_names) + tuple(out_names)
        if part_name is not None:
            bind_names = bind_names + (part_name,)

        def _body(*args):
            operands = list(args)
            if part_name is not None:
                operands.append(bass2jax.partition_id_tensor())
            outs = bass2jax._bass_exec_p.bind(
                *operands,
                out_avals=tuple(out_avals),
                in_names=bind_names,
                out_names=tuple(out_names),
                lowering_input_output_aliases=(),
                sim_require_finite=True,
                sim_require_nnan=True,
                nc=nc,
            )
            return tuple(outs)

        devices = jax.devices()[:B]
        self.mesh = Mesh(np.asarray(devices), ("core",))
        spec = PartitionSpec("core")
        self.sharding = NamedSharding(self.mesh, spec)
        n_in = self.n_params + len(out_names)
        self.fn = jax.jit(
            shard_map(_body, mesh=self.mesh,
                      in_specs=(spec,) * n_in,
                      out_specs=(spec,) * len(out_names),
                      check_rep=False),
            keep_unused=True,
        )

    def _concat_args(self, in_maps):
        args = []
        for i, name in enumerate(self.in_names):
            args.append(np.concatenate([np.asarray(m[name]) for m in in_maps], axis=0))
        for shape, dtype in self.zero_shapes:
            args.append(np.zeros((B * shape[0],) + shape[1:], dtype))
        return args

    def run(self, in_maps):
        out_arrs = self.fn(*self._concat_args(in_maps))
        res = []
        for c in range(B):
            d = {}
            for i, name in enumerate(self.out_names):
                shape = self.out_avals[i].shape
                d[name] = np.asarray(out_arrs[i]).reshape((B,) + shape)[c]
            res.append(d)
        return res


def _build_from_key(key):
    has_b2, act_t, act_s = key
    return _build(has_b2=has_b2, act_t=act_t, act_s=act_s)


def _get_runner(key) -> _Runner:
    ck = ("runner", key)
    if ck not in _CACHE:
        _CACHE[ck] = _Runner(_build_from_key(key))
    return _CACHE[ck]
